# revision 5
# baseline (speedup 1.0000x reference)
"""DGCNN (4x GCNConv + sort-pool + MLP) on 8 trn2 NeuronCores.

Graph-parallel sharding (ranks 0-3: 13 graphs, 4-7: 12). Per layer the
dinv-scaled feature table is AllGathered to DRAM (layer-0 table comes
precomputed from the host), in-edge rows are fetched with big batched
dma_gather instructions (int16 indices; two overlapping 32768-row windows
cover the 53248-row table), reduced per 128-dst slab with strided DVE
tensor_reduce, scaled, transposed on PE and pushed through the 64x64
feature transform + tanh. Sort-pool via max8/max_index/match_replace,
classifier on PE.
"""
import os
import numpy as np

N = 50000
G = 100
NPG = 500
E = 800000
F = 64
K_TOP = 15
CAT = 193
NCORES = 8
SHARD = 6656
NROUND = SHARD // 128          # 52
A_SLOTS = 4096                 # first half of each core's shard (all real+96 zeros)
B_SLOTS = SHARD - A_SLOTS      # 2560
NTOT = SHARD * NCORES          # 53248
WIN = 32768                    # dma_gather int16 window
HI_BASE = NTOT - WIN           # 20480
ZBASE = 4000                   # slots 4000:4096 are reserved zero rows (window lo)
CHUNK_COLS = 32                # gather instruction budget (32*128 = 4096 idxs)
BN_EPS = 1e-5

GRAPHS_PER_CORE = [13, 13, 13, 13, 12, 12, 12, 12]
GSTART = np.concatenate([[0], np.cumsum(GRAPHS_PER_CORE)])

_CACHE = {}


def _slot_base(g_local):
    return g_local * NPG if g_local < 8 else A_SLOTS + (g_local - 8) * NPG


def _table_row(rank, slot):
    """Table layout: [8 cores x slots 0:4096][8 cores x slots 4096:6656]."""
    a = slot < A_SLOTS
    return np.where(a, rank * A_SLOTS + slot,
                    NCORES * A_SLOTS + rank * B_SLOTS + (slot - A_SLOTS))


def _wrap_idx16(flat):
    """dma_gather index layout: idx i -> (partition i%16, col i//16), tiled x8."""
    n = len(flat)
    assert n % 16 == 0
    arr = np.asarray(flat, np.int16).reshape(n // 16, 16).T.copy()
    return np.tile(arr, (8, 1))


def _prep(x, edge_index):
    """Host-side sharding/index preprocessing. Pure numpy."""
    src = edge_index[0].astype(np.int64)
    dst = edge_index[1].astype(np.int64)

    deg = np.bincount(dst, minlength=N).astype(np.float32) + np.float32(1.0)
    dinv = deg ** np.float32(-0.5)
    indeg = np.bincount(dst, minlength=N).astype(np.int64)

    node_graph = np.arange(N) // NPG
    node_rank = np.searchsorted(GSTART, node_graph, side="right") - 1

    order_in_graph = np.zeros(N, np.int64)
    for g in range(G):
        lo = g * NPG
        o = np.argsort(-indeg[lo:lo + NPG], kind="stable")
        r = np.empty(NPG, np.int64)
        r[o] = np.arange(NPG)
        order_in_graph[lo:lo + NPG] = r
    g_local = node_graph - GSTART[node_rank]
    base = np.where(g_local < 8, g_local * NPG, A_SLOTS + (g_local - 8) * NPG)
    slot_of = base + order_in_graph
    trow = _table_row(node_rank, slot_of)          # global table row per node

    # per-edge window: must-lo trow<HI_BASE, must-hi trow>=WIN, else flexible
    e_rank = node_rank[dst]
    srow = trow[src]

    # per (core, slot) per-window edge lists with greedy lo/hi balance
    d_lo = np.zeros((NCORES, SHARD), np.int64)
    d_hi = np.zeros((NCORES, SHARD), np.int64)
    per_core = []
    for c in range(NCORES):
        m = e_rank == c
        s_r = srow[m]            # src table rows
        dslot = slot_of[dst[m]]
        must_lo = s_r < HI_BASE
        must_hi = s_r >= WIN
        flex = ~must_lo & ~must_hi
        # counts
        dg = np.bincount(dslot, minlength=SHARD)
        lo_c = np.bincount(dslot[must_lo & ~flex], minlength=SHARD)
        hi_c = np.bincount(dslot[must_hi], minlength=SHARD)
        fx_c = dg - lo_c - hi_c
        lo_n = np.maximum(lo_c, np.minimum(lo_c + fx_c, (dg + 1) // 2))
        # order edges per slot: flexible ones split between windows
        o = np.argsort(dslot, kind="stable")
        s_r, dslot_s = s_r[o], dslot[o]
        must_hi_s = must_hi[o]
        flex_s = flex[o]
        per_core.append((s_r, dslot_s, must_hi_s, flex_s, dg, lo_n))
        d_lo[c] = lo_n
        d_hi[c] = dg - lo_n

    rl = d_lo.reshape(NCORES, NROUND, 128).max(axis=(0, 2))
    rh = d_hi.reshape(NCORES, NROUND, 128).max(axis=(0, 2))
    Dlo = rl.astype(np.int64)
    Dhi = rh.astype(np.int64)

    # chunk slabs so each window-chunk has <= CHUNK_COLS columns
    def make_chunks(D):
        chunks = []
        cur, cols = [], 0
        for k in range(NROUND):
            d = int(D[k])
            if cur and cols + d > CHUNK_COLS:
                chunks.append((cur, cols))
                cur, cols = [], 0
            cur.append(k)
            cols += d
        if cur:
            chunks.append((cur, cols))
        return chunks

    chunks_lo = make_chunks(Dlo)
    chunks_hi = make_chunks(Dhi)
    cmax = max([c for _, c in chunks_lo + chunks_hi] + [1])

    # per-core window-local index lists (columns ordered chunk by chunk)
    def build_idx(c, D, chunks, is_lo):
        s_r, dslot_s, must_hi_s, flex_s, dg, lo_n = per_core[c]
        # per slot: list of window-local rows
        off = np.concatenate([[0], np.cumsum(dg)])
        total_cols = sum(cc for _, cc in chunks)
        zloc = (c * A_SLOTS + ZBASE) if is_lo else (
            NCORES * A_SLOTS + c * B_SLOTS + B_SLOTS - 1 - HI_BASE)
        out = np.full((128, total_cols), zloc, np.int64)
        colbase = 0
        for slabs, cc in chunks:
            o2 = 0
            for k in slabs:
                d = int(D[k])
                for p in range(128):
                    s = k * 128 + p
                    if dg[s] == 0:
                        o2_ = 0
                    rows = s_r[off[s]:off[s + 1]]
                    mh = must_hi_s[off[s]:off[s + 1]]
                    fx = flex_s[off[s]:off[s + 1]]
                    nlo = int(lo_n[s])
                    # lo gets: all must-lo-nonflex + first part of flex
                    is_must_lo = ~mh & ~fx
                    lo_rows = np.concatenate([rows[is_must_lo], rows[fx]])[:nlo]
                    if is_lo:
                        sel = lo_rows
                    else:
                        n_flex_lo = nlo - int(np.sum(is_must_lo))
                        sel = np.concatenate(
                            [rows[fx][n_flex_lo:], rows[mh]]) - HI_BASE
                        sel2 = np.concatenate([rows[fx][n_flex_lo:], rows[mh]])
                        assert len(sel) == dg[s] - nlo
                    out[p, colbase + o2: colbase + o2 + len(sel)] = sel
                o2 += d
            colbase += cc
        assert (out >= 0).all() and (out < WIN).all()
        # flatten: idx i = col*128 + p
        flat = out.T.reshape(-1)
        return _wrap_idx16(flat)

    idx_lo = [build_idx(c, Dlo, chunks_lo, True) for c in range(NCORES)]
    idx_hi = [build_idx(c, Dhi, chunks_hi, False) for c in range(NCORES)]

    # layer-0 table + node-major u0 + dinv
    u0 = x * dinv[:, None]
    tabA = np.zeros((NCORES * A_SLOTS, F), np.float32)
    tabB = np.zeros((NCORES * B_SLOTS, F), np.float32)
    u0_tab = np.concatenate([tabA, tabB], axis=0)
    u0_tab[trow] = u0
    u_nm0, dinv_nms = [], []
    for c in range(NCORES):
        us = np.zeros((SHARD, F), np.float32)
        dv = np.zeros(SHARD, np.float32)
        nodes = np.arange(NPG * GSTART[c], NPG * GSTART[c + 1])
        us[slot_of[nodes]] = u0[nodes]
        dv[slot_of[nodes]] = dinv[nodes]
        u_nm0.append(np.ascontiguousarray(
            us.reshape(NROUND, 128, F).transpose(1, 0, 2)))
        dinv_nms.append(np.ascontiguousarray(dv.reshape(NROUND, 128).T))

    meta = dict(Dlo=Dlo, Dhi=Dhi, chunks_lo=chunks_lo, chunks_hi=chunks_hi,
                cmax=cmax)
    return meta, idx_lo, idx_hi, u0_tab, u_nm0, dinv_nms


def _pack_classifier(inp):
    Wc0 = np.asarray(inp["Wc0"], np.float32)
    Wc0r = Wc0.reshape(K_TOP, CAT, 256)
    WA = np.ascontiguousarray(Wc0r[:, 0:128, :].transpose(1, 0, 2))
    WB = np.zeros((80, K_TOP, 256), np.float32)
    WB[0:65] = Wc0r[:, 128:193, :].transpose(1, 0, 2)
    sc_full = np.asarray(inp["gamma"], np.float32) * np.float32(
        1.0 / np.sqrt(1.0 + BN_EPS))
    be_full = (np.asarray(inp["beta"], np.float32)
               + np.asarray(inp["bc0"], np.float32) * sc_full)
    sc = np.ascontiguousarray(sc_full.reshape(2, 128).T)
    be = np.ascontiguousarray(be_full.reshape(2, 128).T)
    Wc1 = np.asarray(inp["Wc1"], np.float32)
    Wc1s = np.ascontiguousarray(Wc1.reshape(2, 128, 128).transpose(1, 0, 2))
    gbase = np.array([[_slot_base(g)] for g in range(13)], np.float32)
    return {
        "WA": WA, "WB": WB, "sc": sc, "be": be, "Wc1s": Wc1s,
        "bc1": np.asarray(inp["bc1"], np.float32).reshape(128, 1),
        "Wc2": np.asarray(inp["Wc2"], np.float32),
        "bc2": np.asarray(inp["bc2"], np.float32).reshape(64, 1),
        "Wc3": np.asarray(inp["Wc3"], np.float32),
        "bc3": np.asarray(inp["bc3"], np.float32).reshape(1, 1),
        "gbase": gbase,
    }


def _build(meta):
    import concourse.bass as bass
    import concourse.bacc as bacc
    import concourse.mybir as mybir
    from concourse import tile
    from concourse.masks import make_identity

    f32 = mybir.dt.float32
    AF = mybir.ActivationFunctionType
    Dlo, Dhi = meta["Dlo"], meta["Dhi"]
    chunks_lo, chunks_hi = meta["chunks_lo"], meta["chunks_hi"]
    cmax = meta["cmax"]
    ncl = sum(c for _, c in chunks_lo)
    nch = sum(c for _, c in chunks_hi)

    nc = bacc.Bacc("TRN2", target_bir_lowering=False, debug=False,
                   num_devices=NCORES)

    u_nm0_d = nc.dram_tensor("u_nm0", [128, NROUND, F], f32, kind="ExternalInput")
    ilo_d = nc.dram_tensor("idx_lo", [128, ncl * 8], mybir.dt.int16, kind="ExternalInput")
    ihi_d = nc.dram_tensor("idx_hi", [128, nch * 8], mybir.dt.int16, kind="ExternalInput")
    dinv_d = nc.dram_tensor("dinv_nm", [128, NROUND], f32, kind="ExternalInput")
    u0tab_d = nc.dram_tensor("u0_tab", [NTOT, F], f32, kind="ExternalInput")
    W_d = [nc.dram_tensor(f"W{i}", [F, F if i < 3 else 1], f32, kind="ExternalInput") for i in range(4)]
    b_d = [nc.dram_tensor(f"b{i}", [F if i < 3 else 1, 1], f32, kind="ExternalInput") for i in range(4)]
    WA_d = nc.dram_tensor("WA", [128, K_TOP, 256], f32, kind="ExternalInput")
    WB_d = nc.dram_tensor("WB", [80, K_TOP, 256], f32, kind="ExternalInput")
    sc_d = nc.dram_tensor("sc", [128, 2], f32, kind="ExternalInput")
    be_d = nc.dram_tensor("be", [128, 2], f32, kind="ExternalInput")
    Wc1_d = nc.dram_tensor("Wc1s", [128, 2, 128], f32, kind="ExternalInput")
    bc1_d = nc.dram_tensor("bc1", [128, 1], f32, kind="ExternalInput")
    Wc2_d = nc.dram_tensor("Wc2", [128, F], f32, kind="ExternalInput")
    bc2_d = nc.dram_tensor("bc2", [F, 1], f32, kind="ExternalInput")
    Wc3_d = nc.dram_tensor("Wc3", [F, 1], f32, kind="ExternalInput")
    bc3_d = nc.dram_tensor("bc3", [1, 1], f32, kind="ExternalInput")
    gb_d = nc.dram_tensor("gbase", [13, 1], f32, kind="ExternalInput")
    out_d = nc.dram_tensor("out", [1, 13], f32, kind="ExternalOutput")
    h3dbg_d = nc.dram_tensor("h3dbg", [1, SHARD], f32, kind="ExternalOutput")

    bounceA = nc.dram_tensor("bounceA", [A_SLOTS, F], f32, kind="Internal")
    bounceB = nc.dram_tensor("bounceB", [B_SLOTS, F], f32, kind="Internal")
    u_all = nc.dram_tensor("u_all", [NTOT, F], f32, kind="Internal",
                           addr_space="Shared")
    d6656 = nc.dram_tensor("d6656", [1, SHARD], f32, kind="ExternalOutput")
    d208 = nc.dram_tensor("d208", [1, 208], mybir.dt.int16, kind="Internal")

    with tile.TileContext(nc) as tc:
        with (
            tc.tile_pool(name="persist", bufs=1) as pp,
            tc.tile_pool(name="glo", bufs=2) as gp_lo,
            tc.tile_pool(name="ghi", bufs=2) as gp_hi,
            tc.tile_pool(name="sfm", bufs=2) as sp,
            tc.tile_pool(name="psum_t", bufs=3, space="PSUM") as ps_t,
            tc.tile_pool(name="psum_w", bufs=2, space="PSUM") as ps_w,
        ):
            ident = pp.tile([128, 128], f32)
            make_identity(nc, ident[:])
            ilo = pp.tile([128, ncl * 8], mybir.dt.int16)
            nc.sync.dma_start(ilo[:], ilo_d[:])
            ihi = pp.tile([128, nch * 8], mybir.dt.int16)
            nc.sync.dma_start(ihi[:], ihi_d[:])
            dinv = pp.tile([128, NROUND], f32)
            nc.sync.dma_start(dinv[:], dinv_d[:])
            Ws, bs = [], []
            for i in range(4):
                w = pp.tile([F, F if i < 3 else 1], f32, name=f"W{i}s")
                nc.sync.dma_start(w[:], W_d[i][:])
                Ws.append(w)
                bb = pp.tile([F if i < 3 else 1, 1], f32, name=f"b{i}s")
                nc.sync.dma_start(bb[:], b_d[i][:])
                bs.append(bb)
            hcatA = pp.tile([128, SHARD], f32)
            hcatB = pp.tile([80, SHARD], f32)
            u_nm = pp.tile([128, NROUND, F], f32, name="u_nm")
            nc.sync.dma_start(u_nm[:], u_nm0_d[:])
            fold_lo = pp.tile([128, NROUND, F], f32, name="fold_lo")
            fold_hi = pp.tile([128, NROUND, F], f32, name="fold_hi")
            s_nm = fold_hi

            for layer in range(4):
                tab = u0tab_d if layer == 0 else u_all

                # ---- batched gathers + per-slab strided reduce ----
                for (chunks, D, idxt, gp, fold, lo) in (
                    (chunks_lo, Dlo, ilo, gp_lo, fold_lo, True),
                    (chunks_hi, Dhi, ihi, gp_hi, fold_hi, False),
                ):
                    view = tab[0:WIN, :] if lo else tab[HI_BASE:NTOT, :]
                    colbase = 0
                    for ci, (slabs, cols) in enumerate(chunks):
                        if cols == 0:
                            continue
                        buf = gp.tile([128, cmax, F], f32,
                                      tag="glo" if lo else "ghi",
                                      name=f"g{layer}_{int(lo)}_{ci}")
                        nidx = 128 * cols
                        nc.gpsimd.dma_gather(
                            buf[:, 0:cols, :], view,
                            idxt[:, colbase * 8:(colbase + cols) * 8],
                            nidx, nidx, F, single_packet=False)
                        o = 0
                        for k in slabs:
                            d = int(D[k])
                            if d == 0:
                                nc.vector.memset(fold[:, k, :], 0.0)
                                continue
                            nc.vector.tensor_reduce(
                                out=fold[:, k, :],
                                in_=buf[:, o:o + d, :].rearrange("p d f -> p f d"),
                                axis=mybir.AxisListType.X,
                                op=mybir.AluOpType.add)
                            o += d
                        colbase += cols

                # ---- s = (fold_lo + fold_hi + u) * dinv ----
                nc.vector.tensor_tensor(
                    out=fold_hi[:], in0=fold_lo[:], in1=fold_hi[:],
                    op=mybir.AluOpType.add)
                nc.vector.tensor_tensor(
                    out=fold_lo[:], in0=fold_hi[:], in1=u_nm[:],
                    op=mybir.AluOpType.add)
                for k in range(NROUND):
                    nc.vector.tensor_scalar(
                        out=s_nm[:, k, :], in0=fold_lo[:, k, :],
                        scalar1=dinv[:, k:k + 1], scalar2=None,
                        op0=mybir.AluOpType.mult)

                # ---- transpose chunks + feature transform (+ staging) ----
                fo = F if layer < 3 else 1
                hout = (hcatA[0:64, :] if layer == 0 else
                        hcatA[64:128, :] if layer == 1 else
                        hcatB[0:64, :] if layer == 2 else
                        hcatB[64:65, :])
                hprev_next = (hcatA[0:64, :] if layer == 0 else
                              hcatA[64:128, :] if layer == 1 else
                              hcatB[0:64, :])
                ident64 = (ident[64:128, 64:128] if layer == 1
                           else ident[0:64, 0:64])
                for t in range(13):
                    sfm = sp.tile([F, 512], f32, tag="sfm", name=f"sf{layer}_{t}")
                    for kk in range(4):
                        k = t * 4 + kk
                        ptile = ps_t.tile([128, 128], f32, tag="tr",
                                          name=f"pt{layer}_{k}")
                        nc.tensor.matmul(ptile[0:F, :], s_nm[:, k, :], ident[:],
                                         is_transpose=True)
                        nc.scalar.activation(
                            sfm[:, kk * 128:(kk + 1) * 128], ptile[0:F, :],
                            AF.Copy)
                    pw = ps_w.tile([fo, 512], f32, tag="pw", name=f"pw{layer}_{t}")
                    nc.tensor.matmul(pw[:], Ws[layer][:], sfm[:],
                                     start=True, stop=True)
                    nc.scalar.activation(
                        hout[:, t * 512:(t + 1) * 512], pw[:], AF.Tanh,
                        bias=bs[layer][:])
                    if layer < 3:
                        # stage u for next layer from this chunk
                        for kk in range(4):
                            k = t * 4 + kk
                            pt2 = ps_t.tile([128, 128], f32, tag="tr",
                                            name=f"ps{layer}_{k}")
                            nc.tensor.matmul(
                                pt2[:, 0:F],
                                hout[:, k * 128:(k + 1) * 128],
                                ident64, is_transpose=True)
                            nc.scalar.activation(
                                u_nm[:, k, :], pt2[:, 0:F], AF.Copy,
                                scale=dinv[:, k:k + 1])

                if layer < 3:
                    nc.sync.dma_start(
                        bounceA.ap().rearrange("(t p) f -> p t f", p=128),
                        u_nm[:, 0:A_SLOTS // 128, :])
                    nc.sync.dma_start(
                        bounceB.ap().rearrange("(t p) f -> p t f", p=128),
                        u_nm[:, A_SLOTS // 128:NROUND, :])
                    nc.gpsimd.collective_compute(
                        "AllGather", mybir.AluOpType.bypass,
                        replica_groups=[list(range(NCORES))],
                        ins=[bounceA.ap()],
                        outs=[u_all.ap()[0:NCORES * A_SLOTS, :]])
                    nc.gpsimd.collective_compute(
                        "AllGather", mybir.AluOpType.bypass,
                        replica_groups=[list(range(NCORES))],
                        ins=[bounceB.ap()],
                        outs=[u_all.ap()[NCORES * A_SLOTS:NTOT, :]])

            # ================= sort-pool + classifier =================
            with tc.tile_pool(name="poolp", bufs=1) as qp:
                h3 = hcatB[64:65, :]
                nc.sync.dma_start(d6656.ap(), h3)
                h3g = qp.tile([13, NPG], f32)
                nc.sync.dma_start(
                    h3g[0:8, :],
                    d6656.ap()[:, 0:8 * NPG].rearrange(
                        "one (g i) -> (one g) i", g=8))
                nc.sync.dma_start(
                    h3g[8:13, :],
                    d6656.ap()[:, A_SLOTS:A_SLOTS + 5 * NPG].rearrange(
                        "one (g i) -> (one g) i", g=5))

                m8a = qp.tile([13, 8], f32)
                i8a = qp.tile([13, 8], mybir.dt.uint32)
                nc.vector.max(m8a[:], h3g[:])
                nc.vector.max_index(i8a[:], m8a[:], h3g[:])
                h3m = qp.tile([13, NPG], f32)
                nc.vector.match_replace(h3m[:], m8a[:], h3g[:], -2.0)
                m8b = qp.tile([13, 8], f32)
                i8b = qp.tile([13, 8], mybir.dt.uint32)
                nc.vector.max(m8b[:], h3m[:])
                nc.vector.max_index(i8b[:], m8b[:], h3m[:])

                idx2d = qp.tile([13, 16], f32)
                nc.vector.memset(idx2d[:], 0.0)
                nc.vector.tensor_copy(idx2d[:, 0:8], i8a[:])
                nc.vector.tensor_copy(idx2d[:, 8:15], i8b[:, 0:7])
                gbase = qp.tile([13, 1], f32)
                nc.sync.dma_start(gbase[:], gb_d[:])
                nc.vector.tensor_scalar(
                    out=idx2d[:], in0=idx2d[:], scalar1=gbase[:, 0:1],
                    scalar2=None, op0=mybir.AluOpType.add)
                idx16 = qp.tile([13, 16], mybir.dt.int16)
                nc.vector.tensor_copy(idx16[:], idx2d[:])
                nc.sync.dma_start(
                    d208.ap().rearrange("one (g r) -> (one g) r", g=13),
                    idx16[:])
                idx128 = qp.tile([128, 13], mybir.dt.int16)
                for kk in range(8):
                    nc.sync.dma_start(
                        idx128[kk * 16:(kk + 1) * 16, :],
                        d208.ap().rearrange("one (s p) -> (one p) s", p=16))

                poolA = qp.tile([128, 208], f32)
                nc.gpsimd.ap_gather(poolA[:], hcatA[:], idx128[:],
                                    channels=128, num_elems=SHARD, d=1,
                                    num_idxs=208)
                poolB = qp.tile([80, 208], f32)
                nc.gpsimd.ap_gather(poolB[:], hcatB[:], idx128[0:80, :],
                                    channels=80, num_elems=SHARD, d=1,
                                    num_idxs=208)

                WA = qp.tile([128, K_TOP, 256], f32)
                nc.sync.dma_start(WA[:], WA_d[:])
                WB = qp.tile([80, K_TOP, 256], f32)
                nc.sync.dma_start(WB[:], WB_d[:])
                sc = qp.tile([128, 2], f32)
                nc.sync.dma_start(sc[:], sc_d[:])
                be = qp.tile([128, 2], f32)
                nc.sync.dma_start(be[:], be_d[:])
                Wc1s = qp.tile([128, 2, 128], f32)
                nc.sync.dma_start(Wc1s[:], Wc1_d[:])
                bc1 = qp.tile([128, 1], f32)
                nc.sync.dma_start(bc1[:], bc1_d[:])
                Wc2 = qp.tile([128, F], f32)
                nc.sync.dma_start(Wc2[:], Wc2_d[:])
                bc2 = qp.tile([F, 1], f32)
                nc.sync.dma_start(bc2[:], bc2_d[:])
                Wc3 = qp.tile([F, 1], f32)
                nc.sync.dma_start(Wc3[:], Wc3_d[:])
                bc3 = qp.tile([1, 1], f32)
                nc.sync.dma_start(bc3[:], bc3_d[:])

                z1 = []
                for mh in range(2):
                    pz = ps_w.tile([128, 13], f32, tag="pw", name=f"pz{mh}")
                    first = True
                    for r in range(K_TOP):
                        nc.tensor.matmul(
                            pz[:], WA[:, r, mh * 128:(mh + 1) * 128],
                            poolA[:, r:r + 16 * 12 + 1:16],
                            start=first, stop=False)
                        first = False
                        nc.tensor.matmul(
                            pz[:], WB[0:65, r, mh * 128:(mh + 1) * 128],
                            poolB[0:65, r:r + 16 * 12 + 1:16],
                            start=False, stop=(r == K_TOP - 1))
                    zz = qp.tile([128, 13], f32, tag=f"z1_{mh}", name=f"z1_{mh}")
                    nc.scalar.activation(zz[:], pz[:], AF.Relu,
                                         bias=be[:, mh:mh + 1],
                                         scale=sc[:, mh:mh + 1])
                    z1.append(zz)
                pz2 = ps_w.tile([128, 13], f32, tag="pw", name="pz2")
                nc.tensor.matmul(pz2[:], Wc1s[:, 0, :], z1[0][:],
                                 start=True, stop=False)
                nc.tensor.matmul(pz2[:], Wc1s[:, 1, :], z1[1][:],
                                 start=False, stop=True)
                z2 = qp.tile([128, 13], f32)
                nc.scalar.activation(z2[:], pz2[:], AF.Relu, bias=bc1[:])
                pz3 = ps_w.tile([F, 13], f32, tag="pw", name="pz3")
                nc.tensor.matmul(pz3[:], Wc2[:], z2[:], start=True, stop=True)
                z3 = qp.tile([F, 13], f32)
                nc.scalar.activation(z3[:], pz3[:], AF.Relu, bias=bc2[:])
                pz4 = ps_w.tile([1, 13], f32, tag="pw", name="pz4")
                nc.tensor.matmul(pz4[:], Wc3[:], z3[:], start=True, stop=True)
                zf = qp.tile([1, 13], f32)
                nc.vector.tensor_scalar(out=zf[:], in0=pz4[:],
                                        scalar1=bc3[0:1, 0:1], scalar2=None,
                                        op0=mybir.AluOpType.add)
                nc.sync.dma_start(out_d[:], zf[:])
                nc.sync.dma_start(h3dbg_d[:], h3)

    nc.compile()
    return nc


def kernel(**inputs):
    from concourse import bass_utils

    x = np.asarray(inputs["x"], np.float32)
    edge_index = np.asarray(inputs["edge_index"])

    meta, idx_lo, idx_hi, u0_tab, u_nm0, dinv_nms = _prep(x, edge_index)
    key = ("prog", tuple(meta["Dlo"]), tuple(meta["Dhi"]))
    if key in _CACHE:
        nc = _CACHE[key]
    else:
        nc = _build(meta)
        _CACHE[key] = nc

    cw = _pack_classifier(inputs)
    in_maps = []
    for c in range(NCORES):
        m = {
            "u_nm0": u_nm0[c],
            "idx_lo": idx_lo[c],
            "idx_hi": idx_hi[c],
            "dinv_nm": dinv_nms[c],
            "u0_tab": u0_tab,
            "WA": cw["WA"], "WB": cw["WB"], "sc": cw["sc"], "be": cw["be"],
            "Wc1s": cw["Wc1s"], "bc1": cw["bc1"], "Wc2": cw["Wc2"],
            "bc2": cw["bc2"], "Wc3": cw["Wc3"], "bc3": cw["bc3"],
            "gbase": cw["gbase"],
        }
        for i in range(4):
            m[f"W{i}"] = np.asarray(inputs[f"W{i}"], np.float32).reshape(
                F, F if i < 3 else 1)
            m[f"b{i}"] = np.asarray(inputs[f"b{i}"], np.float32).reshape(
                F if i < 3 else 1, 1)
        in_maps.append(m)

    trace = os.environ.get("KERNEL_TRACE", "0") == "1"
    kwargs = {}
    if trace:
        import sys, types
        if "antenv.axon_hooks" not in sys.modules:
            sys.path.insert(0, "/root/.axon_site")
            from trn_agent_boot.trn_boot import _ntff_profile_via_ctypes
            mm = types.ModuleType("antenv.axon_hooks")
            mm.get_axon_ntff_profile_hook = (
                lambda: _ntff_profile_via_ctypes("/opt/axon/libaxon_pjrt.so"))
            sys.modules["antenv.axon_hooks"] = mm
        import tempfile
        kwargs = dict(trace=True, tmpdir=tempfile.mkdtemp())

    res = bass_utils.run_bass_kernel_spmd(
        nc, in_maps, core_ids=list(range(NCORES)), **kwargs)

    global LAST_EXEC_NS, LAST_H3
    LAST_EXEC_NS = res.exec_time_ns
    LAST_H3 = [res.results[c]["h3dbg"] for c in range(NCORES)]

    out = np.zeros((G, 1), np.float32)
    for c in range(NCORES):
        ngr = GRAPHS_PER_CORE[c]
        out[GSTART[c]:GSTART[c] + ngr, 0] = res.results[c]["out"][0, :ngr]
    return out


LAST_EXEC_NS = None
LAST_H3 = None


# revision 6
# speedup vs baseline: 1.1242x; 1.1242x over previous
"""DGCNN (4x GCNConv + sort-pool + MLP) on 8 trn2 NeuronCores.

Graph-parallel sharding (ranks 0-3: 13 graphs, 4-7: 12). Per layer the
dinv-scaled feature table is AllGathered to DRAM (layer-0 table comes
precomputed from the host), in-edge rows are fetched with big batched
dma_gather instructions (int16 indices; two overlapping 32768-row windows
cover the 53248-row table), reduced per 128-dst slab with strided DVE
tensor_reduce, scaled, transposed on PE and pushed through the 64x64
feature transform + tanh. Sort-pool via max8/max_index/match_replace,
classifier on PE.
"""
import os
import numpy as np

N = 50000
G = 100
NPG = 500
E = 800000
F = 64
K_TOP = 15
CAT = 193
NCORES = 8
SHARD = 6656
NROUND = SHARD // 128          # 52
A_SLOTS = 4096                 # first half of each core's shard (all real+96 zeros)
B_SLOTS = SHARD - A_SLOTS      # 2560
NTOT = SHARD * NCORES          # 53248
WIN = 32768                    # dma_gather int16 window
HI_BASE = NTOT - WIN           # 20480
ZBASE = 4000                   # slots 4000:4096 are reserved zero rows (window lo)
CHUNK_COLS = 32                # gather instruction budget (32*128 = 4096 idxs)
BN_EPS = 1e-5

GRAPHS_PER_CORE = [13, 13, 13, 13, 12, 12, 12, 12]
GSTART = np.concatenate([[0], np.cumsum(GRAPHS_PER_CORE)])

_CACHE = {}


def _slot_base(g_local):
    return g_local * NPG if g_local < 8 else A_SLOTS + (g_local - 8) * NPG


def _table_row(rank, slot):
    """Table layout: [8 cores x slots 0:4096][8 cores x slots 4096:6656]."""
    a = slot < A_SLOTS
    return np.where(a, rank * A_SLOTS + slot,
                    NCORES * A_SLOTS + rank * B_SLOTS + (slot - A_SLOTS))


def _wrap_idx16(flat):
    """dma_gather index layout: idx i -> (partition i%16, col i//16), tiled x8."""
    n = len(flat)
    assert n % 16 == 0
    arr = np.asarray(flat, np.int16).reshape(n // 16, 16).T.copy()
    return np.tile(arr, (8, 1))


def _prep(x, edge_index):
    """Host-side sharding/index preprocessing. Pure numpy."""
    src = edge_index[0].astype(np.int64)
    dst = edge_index[1].astype(np.int64)

    deg = np.bincount(dst, minlength=N).astype(np.float32) + np.float32(1.0)
    dinv = deg ** np.float32(-0.5)
    indeg = np.bincount(dst, minlength=N).astype(np.int64)

    node_graph = np.arange(N) // NPG
    node_rank = np.searchsorted(GSTART, node_graph, side="right") - 1

    order_in_graph = np.zeros(N, np.int64)
    for g in range(G):
        lo = g * NPG
        o = np.argsort(-indeg[lo:lo + NPG], kind="stable")
        r = np.empty(NPG, np.int64)
        r[o] = np.arange(NPG)
        order_in_graph[lo:lo + NPG] = r
    g_local = node_graph - GSTART[node_rank]
    base = np.where(g_local < 8, g_local * NPG, A_SLOTS + (g_local - 8) * NPG)
    slot_of = base + order_in_graph
    trow = _table_row(node_rank, slot_of)          # global table row per node

    # per-edge window: must-lo trow<HI_BASE, must-hi trow>=WIN, else flexible
    e_rank = node_rank[dst]
    srow = trow[src]

    # per (core, slot) per-window edge lists with greedy lo/hi balance
    d_lo = np.zeros((NCORES, SHARD), np.int64)
    d_hi = np.zeros((NCORES, SHARD), np.int64)
    per_core = []
    for c in range(NCORES):
        m = e_rank == c
        s_r = srow[m]            # src table rows
        dslot = slot_of[dst[m]]
        must_lo = s_r < HI_BASE
        must_hi = s_r >= WIN
        flex = ~must_lo & ~must_hi
        # counts
        dg = np.bincount(dslot, minlength=SHARD)
        lo_c = np.bincount(dslot[must_lo & ~flex], minlength=SHARD)
        hi_c = np.bincount(dslot[must_hi], minlength=SHARD)
        fx_c = dg - lo_c - hi_c
        lo_n = np.maximum(lo_c, np.minimum(lo_c + fx_c, (dg + 1) // 2))
        # order edges per slot: flexible ones split between windows
        o = np.argsort(dslot, kind="stable")
        s_r, dslot_s = s_r[o], dslot[o]
        must_hi_s = must_hi[o]
        flex_s = flex[o]
        per_core.append((s_r, dslot_s, must_hi_s, flex_s, dg, lo_n))
        d_lo[c] = lo_n
        d_hi[c] = dg - lo_n

    rl = d_lo.reshape(NCORES, NROUND, 128).max(axis=(0, 2))
    rh = d_hi.reshape(NCORES, NROUND, 128).max(axis=(0, 2))
    Dlo = rl.astype(np.int64)
    Dhi = rh.astype(np.int64)

    # chunk slabs so each window-chunk has <= CHUNK_COLS columns
    def make_chunks(D):
        chunks = []
        cur, cols = [], 0
        for k in range(NROUND):
            d = int(D[k])
            if cur and cols + d > CHUNK_COLS:
                chunks.append((cur, cols))
                cur, cols = [], 0
            cur.append(k)
            cols += d
        if cur:
            chunks.append((cur, cols))
        return chunks

    chunks_lo = make_chunks(Dlo)
    chunks_hi = make_chunks(Dhi)
    cmax = max([c for _, c in chunks_lo + chunks_hi] + [1])

    # per-core window-local index lists (columns ordered chunk by chunk)
    def build_idx(c, D, chunks, is_lo):
        s_r, dslot_s, must_hi_s, flex_s, dg, lo_n = per_core[c]
        # per slot: list of window-local rows
        off = np.concatenate([[0], np.cumsum(dg)])
        total_cols = sum(cc for _, cc in chunks)
        zloc = (c * A_SLOTS + ZBASE) if is_lo else (
            NCORES * A_SLOTS + c * B_SLOTS + B_SLOTS - 1 - HI_BASE)
        out = np.full((128, total_cols), zloc, np.int64)
        colbase = 0
        for slabs, cc in chunks:
            o2 = 0
            for k in slabs:
                d = int(D[k])
                for p in range(128):
                    s = k * 128 + p
                    if dg[s] == 0:
                        o2_ = 0
                    rows = s_r[off[s]:off[s + 1]]
                    mh = must_hi_s[off[s]:off[s + 1]]
                    fx = flex_s[off[s]:off[s + 1]]
                    nlo = int(lo_n[s])
                    # lo gets: all must-lo-nonflex + first part of flex
                    is_must_lo = ~mh & ~fx
                    lo_rows = np.concatenate([rows[is_must_lo], rows[fx]])[:nlo]
                    if is_lo:
                        sel = lo_rows
                    else:
                        n_flex_lo = nlo - int(np.sum(is_must_lo))
                        sel = np.concatenate(
                            [rows[fx][n_flex_lo:], rows[mh]]) - HI_BASE
                        sel2 = np.concatenate([rows[fx][n_flex_lo:], rows[mh]])
                        assert len(sel) == dg[s] - nlo
                    out[p, colbase + o2: colbase + o2 + len(sel)] = sel
                o2 += d
            colbase += cc
        assert (out >= 0).all() and (out < WIN).all()
        # flatten: idx i = col*128 + p
        flat = out.T.reshape(-1)
        return _wrap_idx16(flat)

    idx_lo = [build_idx(c, Dlo, chunks_lo, True) for c in range(NCORES)]
    idx_hi = [build_idx(c, Dhi, chunks_hi, False) for c in range(NCORES)]

    # layer-0 table + node-major u0 + dinv
    u0 = x * dinv[:, None]
    tabA = np.zeros((NCORES * A_SLOTS, F), np.float32)
    tabB = np.zeros((NCORES * B_SLOTS, F), np.float32)
    u0_tab = np.concatenate([tabA, tabB], axis=0)
    u0_tab[trow] = u0
    u_nm0, dinv_nms = [], []
    for c in range(NCORES):
        us = np.zeros((SHARD, F), np.float32)
        dv = np.zeros(SHARD, np.float32)
        nodes = np.arange(NPG * GSTART[c], NPG * GSTART[c + 1])
        us[slot_of[nodes]] = u0[nodes]
        dv[slot_of[nodes]] = dinv[nodes]
        u_nm0.append(np.ascontiguousarray(
            us.reshape(NROUND, 128, F).transpose(1, 0, 2)))
        dinv_nms.append(np.ascontiguousarray(dv.reshape(NROUND, 128).T))

    meta = dict(Dlo=Dlo, Dhi=Dhi, chunks_lo=chunks_lo, chunks_hi=chunks_hi,
                cmax=cmax)
    return meta, idx_lo, idx_hi, u0_tab, u_nm0, dinv_nms


def _pack_classifier(inp):
    Wc0 = np.asarray(inp["Wc0"], np.float32)
    Wc0r = Wc0.reshape(K_TOP, CAT, 256)
    WA = np.ascontiguousarray(Wc0r[:, 0:128, :].transpose(1, 0, 2))
    WB = np.zeros((80, K_TOP, 256), np.float32)
    WB[0:65] = Wc0r[:, 128:193, :].transpose(1, 0, 2)
    sc_full = np.asarray(inp["gamma"], np.float32) * np.float32(
        1.0 / np.sqrt(1.0 + BN_EPS))
    be_full = (np.asarray(inp["beta"], np.float32)
               + np.asarray(inp["bc0"], np.float32) * sc_full)
    sc = np.ascontiguousarray(sc_full.reshape(2, 128).T)
    be = np.ascontiguousarray(be_full.reshape(2, 128).T)
    Wc1 = np.asarray(inp["Wc1"], np.float32)
    Wc1s = np.ascontiguousarray(Wc1.reshape(2, 128, 128).transpose(1, 0, 2))
    gbase = np.array([[_slot_base(g)] for g in range(13)], np.float32)
    return {
        "WA": WA, "WB": WB, "sc": sc, "be": be, "Wc1s": Wc1s,
        "bc1": np.asarray(inp["bc1"], np.float32).reshape(128, 1),
        "Wc2": np.asarray(inp["Wc2"], np.float32),
        "bc2": np.asarray(inp["bc2"], np.float32).reshape(64, 1),
        "Wc3": np.asarray(inp["Wc3"], np.float32),
        "bc3": np.asarray(inp["bc3"], np.float32).reshape(1, 1),
        "gbase": gbase,
    }


def _build(meta):
    import concourse.bass as bass
    import concourse.bacc as bacc
    import concourse.mybir as mybir
    from concourse import tile
    from concourse.masks import make_identity

    f32 = mybir.dt.float32
    AF = mybir.ActivationFunctionType
    Dlo, Dhi = meta["Dlo"], meta["Dhi"]
    chunks_lo, chunks_hi = meta["chunks_lo"], meta["chunks_hi"]
    cmax = meta["cmax"]
    ncl = sum(c for _, c in chunks_lo)
    nch = sum(c for _, c in chunks_hi)

    nc = bacc.Bacc("TRN2", target_bir_lowering=False, debug=False,
                   num_devices=NCORES, num_swdge_queues=4)

    u_nm0_d = nc.dram_tensor("u_nm0", [128, NROUND, F], f32, kind="ExternalInput")
    ilo_d = nc.dram_tensor("idx_lo", [128, ncl * 8], mybir.dt.int16, kind="ExternalInput")
    ihi_d = nc.dram_tensor("idx_hi", [128, nch * 8], mybir.dt.int16, kind="ExternalInput")
    dinv_d = nc.dram_tensor("dinv_nm", [128, NROUND], f32, kind="ExternalInput")
    u0tab_d = nc.dram_tensor("u0_tab", [NTOT, F], f32, kind="ExternalInput")
    W_d = [nc.dram_tensor(f"W{i}", [F, F if i < 3 else 1], f32, kind="ExternalInput") for i in range(4)]
    b_d = [nc.dram_tensor(f"b{i}", [F if i < 3 else 1, 1], f32, kind="ExternalInput") for i in range(4)]
    WA_d = nc.dram_tensor("WA", [128, K_TOP, 256], f32, kind="ExternalInput")
    WB_d = nc.dram_tensor("WB", [80, K_TOP, 256], f32, kind="ExternalInput")
    sc_d = nc.dram_tensor("sc", [128, 2], f32, kind="ExternalInput")
    be_d = nc.dram_tensor("be", [128, 2], f32, kind="ExternalInput")
    Wc1_d = nc.dram_tensor("Wc1s", [128, 2, 128], f32, kind="ExternalInput")
    bc1_d = nc.dram_tensor("bc1", [128, 1], f32, kind="ExternalInput")
    Wc2_d = nc.dram_tensor("Wc2", [128, F], f32, kind="ExternalInput")
    bc2_d = nc.dram_tensor("bc2", [F, 1], f32, kind="ExternalInput")
    Wc3_d = nc.dram_tensor("Wc3", [F, 1], f32, kind="ExternalInput")
    bc3_d = nc.dram_tensor("bc3", [1, 1], f32, kind="ExternalInput")
    gb_d = nc.dram_tensor("gbase", [13, 1], f32, kind="ExternalInput")
    out_d = nc.dram_tensor("out", [1, 13], f32, kind="ExternalOutput")
    h3dbg_d = nc.dram_tensor("h3dbg", [1, SHARD], f32, kind="ExternalOutput")

    bounceA = nc.dram_tensor("bounceA", [A_SLOTS, F], f32, kind="Internal")
    bounceB = nc.dram_tensor("bounceB", [B_SLOTS, F], f32, kind="Internal")
    u_all = nc.dram_tensor("u_all", [NTOT, F], f32, kind="Internal",
                           addr_space="Shared")
    d6656 = nc.dram_tensor("d6656", [1, SHARD], f32, kind="ExternalOutput")
    d208 = nc.dram_tensor("d208", [1, 208], mybir.dt.int16, kind="Internal")

    with tile.TileContext(nc) as tc:
        with (
            tc.tile_pool(name="persist", bufs=1) as pp,
            tc.tile_pool(name="glo", bufs=2) as gp_lo,
            tc.tile_pool(name="ghi", bufs=2) as gp_hi,
            tc.tile_pool(name="sfm", bufs=2) as sp,
            tc.tile_pool(name="psum_t", bufs=3, space="PSUM") as ps_t,
            tc.tile_pool(name="psum_w", bufs=2, space="PSUM") as ps_w,
        ):
            ident = pp.tile([128, 128], f32)
            make_identity(nc, ident[:])
            ilo = pp.tile([128, ncl * 8], mybir.dt.int16)
            nc.sync.dma_start(ilo[:], ilo_d[:])
            ihi = pp.tile([128, nch * 8], mybir.dt.int16)
            nc.sync.dma_start(ihi[:], ihi_d[:])
            dinv = pp.tile([128, NROUND], f32)
            nc.sync.dma_start(dinv[:], dinv_d[:])
            Ws, bs = [], []
            for i in range(4):
                w = pp.tile([F, F if i < 3 else 1], f32, name=f"W{i}s")
                nc.sync.dma_start(w[:], W_d[i][:])
                Ws.append(w)
                bb = pp.tile([F if i < 3 else 1, 1], f32, name=f"b{i}s")
                nc.sync.dma_start(bb[:], b_d[i][:])
                bs.append(bb)
            hcatA = pp.tile([128, SHARD], f32)
            hcatB = pp.tile([80, SHARD], f32)
            u_nm = pp.tile([128, NROUND, F], f32, name="u_nm")
            nc.sync.dma_start(u_nm[:], u_nm0_d[:])
            fold_lo = pp.tile([128, NROUND, F], f32, name="fold_lo")
            fold_hi = pp.tile([128, NROUND, F], f32, name="fold_hi")
            s_nm = fold_hi

            for layer in range(4):
                tab = u0tab_d if layer == 0 else u_all

                # ---- batched gathers + per-slab strided reduce ----
                for (chunks, D, idxt, gp, fold, lo) in (
                    (chunks_lo, Dlo, ilo, gp_lo, fold_lo, True),
                    (chunks_hi, Dhi, ihi, gp_hi, fold_hi, False),
                ):
                    view = tab[0:WIN, :] if lo else tab[HI_BASE:NTOT, :]
                    colbase = 0
                    qn = 0
                    for ci, (slabs, cols) in enumerate(chunks):
                        if cols == 0:
                            continue
                        buf = gp.tile([128, cmax, F], f32,
                                      tag="glo" if lo else "ghi",
                                      name=f"g{layer}_{int(lo)}_{ci}")
                        nidx = 128 * cols
                        nc.gpsimd.dma_gather(
                            buf[:, 0:cols, :], view,
                            idxt[:, colbase * 8:(colbase + cols) * 8],
                            nidx, nidx, F, single_packet=False,
                            queue_num=(ci % 2) * 2 + (0 if lo else 1))
                        o = 0
                        for k in slabs:
                            d = int(D[k])
                            if d == 0:
                                nc.vector.memset(fold[:, k, :], 0.0)
                                continue
                            nc.vector.tensor_reduce(
                                out=fold[:, k, :],
                                in_=buf[:, o:o + d, :].rearrange("p d f -> p f d"),
                                axis=mybir.AxisListType.X,
                                op=mybir.AluOpType.add)
                            o += d
                        colbase += cols

                # ---- s = (fold_lo + fold_hi + u) * dinv ----
                nc.vector.tensor_tensor(
                    out=fold_hi[:], in0=fold_lo[:], in1=fold_hi[:],
                    op=mybir.AluOpType.add)
                nc.vector.tensor_tensor(
                    out=fold_lo[:], in0=fold_hi[:], in1=u_nm[:],
                    op=mybir.AluOpType.add)
                for k in range(NROUND):
                    nc.vector.tensor_scalar(
                        out=s_nm[:, k, :], in0=fold_lo[:, k, :],
                        scalar1=dinv[:, k:k + 1], scalar2=None,
                        op0=mybir.AluOpType.mult)

                # ---- transpose chunks + feature transform (+ staging) ----
                fo = F if layer < 3 else 1
                hout = (hcatA[0:64, :] if layer == 0 else
                        hcatA[64:128, :] if layer == 1 else
                        hcatB[0:64, :] if layer == 2 else
                        hcatB[64:65, :])
                hprev_next = (hcatA[0:64, :] if layer == 0 else
                              hcatA[64:128, :] if layer == 1 else
                              hcatB[0:64, :])
                ident64 = (ident[64:128, 64:128] if layer == 1
                           else ident[0:64, 0:64])
                for t in range(13):
                    sfm = sp.tile([F, 512], f32, tag="sfm", name=f"sf{layer}_{t}")
                    for kk in range(4):
                        k = t * 4 + kk
                        ptile = ps_t.tile([128, 128], f32, tag="tr",
                                          name=f"pt{layer}_{k}")
                        nc.tensor.matmul(ptile[0:F, :], s_nm[:, k, :], ident[:],
                                         is_transpose=True)
                        nc.scalar.activation(
                            sfm[:, kk * 128:(kk + 1) * 128], ptile[0:F, :],
                            AF.Copy)
                    pw = ps_w.tile([fo, 512], f32, tag="pw", name=f"pw{layer}_{t}")
                    nc.tensor.matmul(pw[:], Ws[layer][:], sfm[:],
                                     start=True, stop=True)
                    nc.scalar.activation(
                        hout[:, t * 512:(t + 1) * 512], pw[:], AF.Tanh,
                        bias=bs[layer][:])
                    if layer < 3:
                        # stage u for next layer from this chunk
                        for kk in range(4):
                            k = t * 4 + kk
                            pt2 = ps_t.tile([128, 128], f32, tag="tr",
                                            name=f"ps{layer}_{k}")
                            nc.tensor.matmul(
                                pt2[:, 0:F],
                                hout[:, k * 128:(k + 1) * 128],
                                ident64, is_transpose=True)
                            nc.scalar.activation(
                                u_nm[:, k, :], pt2[:, 0:F], AF.Copy,
                                scale=dinv[:, k:k + 1])

                if layer < 3:
                    nc.sync.dma_start(
                        bounceA.ap().rearrange("(t p) f -> p t f", p=128),
                        u_nm[:, 0:A_SLOTS // 128, :])
                    nc.sync.dma_start(
                        bounceB.ap().rearrange("(t p) f -> p t f", p=128),
                        u_nm[:, A_SLOTS // 128:NROUND, :])
                    nc.gpsimd.collective_compute(
                        "AllGather", mybir.AluOpType.bypass,
                        replica_groups=[list(range(NCORES))],
                        ins=[bounceA.ap()],
                        outs=[u_all.ap()[0:NCORES * A_SLOTS, :]])
                    nc.gpsimd.collective_compute(
                        "AllGather", mybir.AluOpType.bypass,
                        replica_groups=[list(range(NCORES))],
                        ins=[bounceB.ap()],
                        outs=[u_all.ap()[NCORES * A_SLOTS:NTOT, :]])

            # ================= sort-pool + classifier =================
            with tc.tile_pool(name="poolp", bufs=1) as qp:
                h3 = hcatB[64:65, :]
                nc.sync.dma_start(d6656.ap(), h3)
                h3g = qp.tile([13, NPG], f32)
                nc.sync.dma_start(
                    h3g[0:8, :],
                    d6656.ap()[:, 0:8 * NPG].rearrange(
                        "one (g i) -> (one g) i", g=8))
                nc.sync.dma_start(
                    h3g[8:13, :],
                    d6656.ap()[:, A_SLOTS:A_SLOTS + 5 * NPG].rearrange(
                        "one (g i) -> (one g) i", g=5))

                m8a = qp.tile([13, 8], f32)
                i8a = qp.tile([13, 8], mybir.dt.uint32)
                nc.vector.max(m8a[:], h3g[:])
                nc.vector.max_index(i8a[:], m8a[:], h3g[:])
                h3m = qp.tile([13, NPG], f32)
                nc.vector.match_replace(h3m[:], m8a[:], h3g[:], -2.0)
                m8b = qp.tile([13, 8], f32)
                i8b = qp.tile([13, 8], mybir.dt.uint32)
                nc.vector.max(m8b[:], h3m[:])
                nc.vector.max_index(i8b[:], m8b[:], h3m[:])

                idx2d = qp.tile([13, 16], f32)
                nc.vector.memset(idx2d[:], 0.0)
                nc.vector.tensor_copy(idx2d[:, 0:8], i8a[:])
                nc.vector.tensor_copy(idx2d[:, 8:15], i8b[:, 0:7])
                gbase = qp.tile([13, 1], f32)
                nc.sync.dma_start(gbase[:], gb_d[:])
                nc.vector.tensor_scalar(
                    out=idx2d[:], in0=idx2d[:], scalar1=gbase[:, 0:1],
                    scalar2=None, op0=mybir.AluOpType.add)
                idx16 = qp.tile([13, 16], mybir.dt.int16)
                nc.vector.tensor_copy(idx16[:], idx2d[:])
                nc.sync.dma_start(
                    d208.ap().rearrange("one (g r) -> (one g) r", g=13),
                    idx16[:])
                idx128 = qp.tile([128, 13], mybir.dt.int16)
                for kk in range(8):
                    nc.sync.dma_start(
                        idx128[kk * 16:(kk + 1) * 16, :],
                        d208.ap().rearrange("one (s p) -> (one p) s", p=16))

                poolA = qp.tile([128, 208], f32)
                nc.gpsimd.ap_gather(poolA[:], hcatA[:], idx128[:],
                                    channels=128, num_elems=SHARD, d=1,
                                    num_idxs=208)
                poolB = qp.tile([80, 208], f32)
                nc.gpsimd.ap_gather(poolB[:], hcatB[:], idx128[0:80, :],
                                    channels=80, num_elems=SHARD, d=1,
                                    num_idxs=208)

                WA = qp.tile([128, K_TOP, 256], f32)
                nc.sync.dma_start(WA[:], WA_d[:])
                WB = qp.tile([80, K_TOP, 256], f32)
                nc.sync.dma_start(WB[:], WB_d[:])
                sc = qp.tile([128, 2], f32)
                nc.sync.dma_start(sc[:], sc_d[:])
                be = qp.tile([128, 2], f32)
                nc.sync.dma_start(be[:], be_d[:])
                Wc1s = qp.tile([128, 2, 128], f32)
                nc.sync.dma_start(Wc1s[:], Wc1_d[:])
                bc1 = qp.tile([128, 1], f32)
                nc.sync.dma_start(bc1[:], bc1_d[:])
                Wc2 = qp.tile([128, F], f32)
                nc.sync.dma_start(Wc2[:], Wc2_d[:])
                bc2 = qp.tile([F, 1], f32)
                nc.sync.dma_start(bc2[:], bc2_d[:])
                Wc3 = qp.tile([F, 1], f32)
                nc.sync.dma_start(Wc3[:], Wc3_d[:])
                bc3 = qp.tile([1, 1], f32)
                nc.sync.dma_start(bc3[:], bc3_d[:])

                z1 = []
                for mh in range(2):
                    pz = ps_w.tile([128, 13], f32, tag="pw", name=f"pz{mh}")
                    first = True
                    for r in range(K_TOP):
                        nc.tensor.matmul(
                            pz[:], WA[:, r, mh * 128:(mh + 1) * 128],
                            poolA[:, r:r + 16 * 12 + 1:16],
                            start=first, stop=False)
                        first = False
                        nc.tensor.matmul(
                            pz[:], WB[0:65, r, mh * 128:(mh + 1) * 128],
                            poolB[0:65, r:r + 16 * 12 + 1:16],
                            start=False, stop=(r == K_TOP - 1))
                    zz = qp.tile([128, 13], f32, tag=f"z1_{mh}", name=f"z1_{mh}")
                    nc.scalar.activation(zz[:], pz[:], AF.Relu,
                                         bias=be[:, mh:mh + 1],
                                         scale=sc[:, mh:mh + 1])
                    z1.append(zz)
                pz2 = ps_w.tile([128, 13], f32, tag="pw", name="pz2")
                nc.tensor.matmul(pz2[:], Wc1s[:, 0, :], z1[0][:],
                                 start=True, stop=False)
                nc.tensor.matmul(pz2[:], Wc1s[:, 1, :], z1[1][:],
                                 start=False, stop=True)
                z2 = qp.tile([128, 13], f32)
                nc.scalar.activation(z2[:], pz2[:], AF.Relu, bias=bc1[:])
                pz3 = ps_w.tile([F, 13], f32, tag="pw", name="pz3")
                nc.tensor.matmul(pz3[:], Wc2[:], z2[:], start=True, stop=True)
                z3 = qp.tile([F, 13], f32)
                nc.scalar.activation(z3[:], pz3[:], AF.Relu, bias=bc2[:])
                pz4 = ps_w.tile([1, 13], f32, tag="pw", name="pz4")
                nc.tensor.matmul(pz4[:], Wc3[:], z3[:], start=True, stop=True)
                zf = qp.tile([1, 13], f32)
                nc.vector.tensor_scalar(out=zf[:], in0=pz4[:],
                                        scalar1=bc3[0:1, 0:1], scalar2=None,
                                        op0=mybir.AluOpType.add)
                nc.sync.dma_start(out_d[:], zf[:])
                nc.sync.dma_start(h3dbg_d[:], h3)

    nc.compile()
    return nc


def kernel(**inputs):
    from concourse import bass_utils

    x = np.asarray(inputs["x"], np.float32)
    edge_index = np.asarray(inputs["edge_index"])

    meta, idx_lo, idx_hi, u0_tab, u_nm0, dinv_nms = _prep(x, edge_index)
    key = ("prog", tuple(meta["Dlo"]), tuple(meta["Dhi"]))
    if key in _CACHE:
        nc = _CACHE[key]
    else:
        nc = _build(meta)
        _CACHE[key] = nc

    cw = _pack_classifier(inputs)
    in_maps = []
    for c in range(NCORES):
        m = {
            "u_nm0": u_nm0[c],
            "idx_lo": idx_lo[c],
            "idx_hi": idx_hi[c],
            "dinv_nm": dinv_nms[c],
            "u0_tab": u0_tab,
            "WA": cw["WA"], "WB": cw["WB"], "sc": cw["sc"], "be": cw["be"],
            "Wc1s": cw["Wc1s"], "bc1": cw["bc1"], "Wc2": cw["Wc2"],
            "bc2": cw["bc2"], "Wc3": cw["Wc3"], "bc3": cw["bc3"],
            "gbase": cw["gbase"],
        }
        for i in range(4):
            m[f"W{i}"] = np.asarray(inputs[f"W{i}"], np.float32).reshape(
                F, F if i < 3 else 1)
            m[f"b{i}"] = np.asarray(inputs[f"b{i}"], np.float32).reshape(
                F if i < 3 else 1, 1)
        in_maps.append(m)

    trace = os.environ.get("KERNEL_TRACE", "0") == "1"
    kwargs = {}
    if trace:
        import sys, types
        if "antenv.axon_hooks" not in sys.modules:
            sys.path.insert(0, "/root/.axon_site")
            from trn_agent_boot.trn_boot import _ntff_profile_via_ctypes
            mm = types.ModuleType("antenv.axon_hooks")
            mm.get_axon_ntff_profile_hook = (
                lambda: _ntff_profile_via_ctypes("/opt/axon/libaxon_pjrt.so"))
            sys.modules["antenv.axon_hooks"] = mm
        import tempfile
        kwargs = dict(trace=True, tmpdir=tempfile.mkdtemp())

    res = bass_utils.run_bass_kernel_spmd(
        nc, in_maps, core_ids=list(range(NCORES)), **kwargs)

    global LAST_EXEC_NS, LAST_H3
    LAST_EXEC_NS = res.exec_time_ns
    LAST_H3 = [res.results[c]["h3dbg"] for c in range(NCORES)]

    out = np.zeros((G, 1), np.float32)
    for c in range(NCORES):
        ngr = GRAPHS_PER_CORE[c]
        out[GSTART[c]:GSTART[c] + ngr, 0] = res.results[c]["out"][0, :ngr]
    return out


LAST_EXEC_NS = None
LAST_H3 = None


# revision 7
# speedup vs baseline: 1.2493x; 1.1113x over previous
"""DGCNN (4x GCNConv + sort-pool + MLP) on 8 trn2 NeuronCores.

Graph-parallel sharding (ranks 0-3: 13 graphs, 4-7: 12). Per layer the
dinv-scaled feature table is AllGathered to DRAM (layer-0 table comes
precomputed from the host), in-edge rows are fetched with big batched
dma_gather instructions (int16 indices; two overlapping 32768-row windows
cover the 53248-row table), reduced per 128-dst slab with strided DVE
tensor_reduce, scaled, transposed on PE and pushed through the 64x64
feature transform + tanh. Sort-pool via max8/max_index/match_replace,
classifier on PE.
"""
import os
import numpy as np

N = 50000
G = 100
NPG = 500
E = 800000
F = 64
K_TOP = 15
CAT = 193
NCORES = 8
SHARD = 6656
NROUND = SHARD // 128          # 52
A_SLOTS = 4096                 # first half of each core's shard (all real+96 zeros)
B_SLOTS = SHARD - A_SLOTS      # 2560
NTOT = SHARD * NCORES          # 53248
WIN = 32768                    # dma_gather int16 window
HI_BASE = NTOT - WIN           # 20480
ZBASE = 4000                   # slots 4000:4096 are reserved zero rows (window lo)
CHUNK_COLS = 24                # gather instruction budget (32*128 = 4096 idxs)
BN_EPS = 1e-5

GRAPHS_PER_CORE = [13, 13, 13, 13, 12, 12, 12, 12]
GSTART = np.concatenate([[0], np.cumsum(GRAPHS_PER_CORE)])

_CACHE = {}


def _slot_base(g_local):
    return g_local * NPG if g_local < 8 else A_SLOTS + (g_local - 8) * NPG


def _table_row(rank, slot):
    """Table layout: [8 cores x slots 0:4096][8 cores x slots 4096:6656]."""
    a = slot < A_SLOTS
    return np.where(a, rank * A_SLOTS + slot,
                    NCORES * A_SLOTS + rank * B_SLOTS + (slot - A_SLOTS))


def _wrap_idx16(flat):
    """dma_gather index layout: idx i -> (partition i%16, col i//16), tiled x8."""
    n = len(flat)
    assert n % 16 == 0
    arr = np.asarray(flat, np.int16).reshape(n // 16, 16).T.copy()
    return np.tile(arr, (8, 1))


def _prep(x, edge_index):
    """Host-side sharding/index preprocessing. Pure numpy."""
    src = edge_index[0].astype(np.int64)
    dst = edge_index[1].astype(np.int64)

    deg = np.bincount(dst, minlength=N).astype(np.float32) + np.float32(1.0)
    dinv = deg ** np.float32(-0.5)
    indeg = np.bincount(dst, minlength=N).astype(np.int64)

    node_graph = np.arange(N) // NPG
    node_rank = np.searchsorted(GSTART, node_graph, side="right") - 1

    order_in_graph = np.zeros(N, np.int64)
    for g in range(G):
        lo = g * NPG
        o = np.argsort(-indeg[lo:lo + NPG], kind="stable")
        r = np.empty(NPG, np.int64)
        r[o] = np.arange(NPG)
        order_in_graph[lo:lo + NPG] = r
    g_local = node_graph - GSTART[node_rank]
    base = np.where(g_local < 8, g_local * NPG, A_SLOTS + (g_local - 8) * NPG)
    slot_of = base + order_in_graph
    trow = _table_row(node_rank, slot_of)          # global table row per node

    # per-edge window: must-lo trow<HI_BASE, must-hi trow>=WIN, else flexible
    e_rank = node_rank[dst]
    srow = trow[src]

    # per (core, slot) per-window edge lists with greedy lo/hi balance
    d_lo = np.zeros((NCORES, SHARD), np.int64)
    d_hi = np.zeros((NCORES, SHARD), np.int64)
    per_core = []
    for c in range(NCORES):
        m = e_rank == c
        s_r = srow[m]            # src table rows
        dslot = slot_of[dst[m]]
        must_lo = s_r < HI_BASE
        must_hi = s_r >= WIN
        flex = ~must_lo & ~must_hi
        # counts
        dg = np.bincount(dslot, minlength=SHARD)
        lo_c = np.bincount(dslot[must_lo & ~flex], minlength=SHARD)
        hi_c = np.bincount(dslot[must_hi], minlength=SHARD)
        fx_c = dg - lo_c - hi_c
        lo_n = np.maximum(lo_c, np.minimum(lo_c + fx_c, (dg + 1) // 2))
        # order edges per slot: flexible ones split between windows
        o = np.argsort(dslot, kind="stable")
        s_r, dslot_s = s_r[o], dslot[o]
        must_hi_s = must_hi[o]
        flex_s = flex[o]
        per_core.append((s_r, dslot_s, must_hi_s, flex_s, dg, lo_n))
        d_lo[c] = lo_n
        d_hi[c] = dg - lo_n

    rl = d_lo.reshape(NCORES, NROUND, 128).max(axis=(0, 2))
    rh = d_hi.reshape(NCORES, NROUND, 128).max(axis=(0, 2))
    Dlo = rl.astype(np.int64)
    Dhi = rh.astype(np.int64)

    # chunk slabs so each window-chunk has <= CHUNK_COLS columns
    def make_chunks(D):
        chunks = []
        cur, cols = [], 0
        for k in range(NROUND):
            d = int(D[k])
            if cur and cols + d > CHUNK_COLS:
                chunks.append((cur, cols))
                cur, cols = [], 0
            cur.append(k)
            cols += d
        if cur:
            chunks.append((cur, cols))
        return chunks

    chunks_lo = make_chunks(Dlo)
    chunks_hi = make_chunks(Dhi)
    cmax = max([c for _, c in chunks_lo + chunks_hi] + [1])

    # per-core window-local index lists (columns ordered chunk by chunk)
    def build_idx(c, D, chunks, is_lo):
        s_r, dslot_s, must_hi_s, flex_s, dg, lo_n = per_core[c]
        # per slot: list of window-local rows
        off = np.concatenate([[0], np.cumsum(dg)])
        total_cols = sum(cc for _, cc in chunks)
        zloc = (c * A_SLOTS + ZBASE) if is_lo else (
            NCORES * A_SLOTS + c * B_SLOTS + B_SLOTS - 1 - HI_BASE)
        out = np.full((128, total_cols), zloc, np.int64)
        colbase = 0
        for slabs, cc in chunks:
            o2 = 0
            for k in slabs:
                d = int(D[k])
                for p in range(128):
                    s = k * 128 + p
                    if dg[s] == 0:
                        o2_ = 0
                    rows = s_r[off[s]:off[s + 1]]
                    mh = must_hi_s[off[s]:off[s + 1]]
                    fx = flex_s[off[s]:off[s + 1]]
                    nlo = int(lo_n[s])
                    # lo gets: all must-lo-nonflex + first part of flex
                    is_must_lo = ~mh & ~fx
                    lo_rows = np.concatenate([rows[is_must_lo], rows[fx]])[:nlo]
                    if is_lo:
                        sel = lo_rows
                    else:
                        n_flex_lo = nlo - int(np.sum(is_must_lo))
                        sel = np.concatenate(
                            [rows[fx][n_flex_lo:], rows[mh]]) - HI_BASE
                        sel2 = np.concatenate([rows[fx][n_flex_lo:], rows[mh]])
                        assert len(sel) == dg[s] - nlo
                    out[p, colbase + o2: colbase + o2 + len(sel)] = sel
                o2 += d
            colbase += cc
        assert (out >= 0).all() and (out < WIN).all()
        # flatten: idx i = col*128 + p
        flat = out.T.reshape(-1)
        return _wrap_idx16(flat)

    idx_lo = [build_idx(c, Dlo, chunks_lo, True) for c in range(NCORES)]
    idx_hi = [build_idx(c, Dhi, chunks_hi, False) for c in range(NCORES)]

    # layer-0 table + node-major u0 + dinv
    u0 = x * dinv[:, None]
    tabA = np.zeros((NCORES * A_SLOTS, F), np.float32)
    tabB = np.zeros((NCORES * B_SLOTS, F), np.float32)
    u0_tab = np.concatenate([tabA, tabB], axis=0)
    u0_tab[trow] = u0
    u_nm0, dinv_nms = [], []
    for c in range(NCORES):
        us = np.zeros((SHARD, F), np.float32)
        dv = np.zeros(SHARD, np.float32)
        nodes = np.arange(NPG * GSTART[c], NPG * GSTART[c + 1])
        us[slot_of[nodes]] = u0[nodes]
        dv[slot_of[nodes]] = dinv[nodes]
        u_nm0.append(np.ascontiguousarray(
            us.reshape(NROUND, 128, F).transpose(1, 0, 2)))
        dinv_nms.append(np.ascontiguousarray(dv.reshape(NROUND, 128).T))

    meta = dict(Dlo=Dlo, Dhi=Dhi, chunks_lo=chunks_lo, chunks_hi=chunks_hi,
                cmax=cmax)
    return meta, idx_lo, idx_hi, u0_tab, u_nm0, dinv_nms


def _pack_classifier(inp):
    Wc0 = np.asarray(inp["Wc0"], np.float32)
    Wc0r = Wc0.reshape(K_TOP, CAT, 256)
    WA = np.ascontiguousarray(Wc0r[:, 0:128, :].transpose(1, 0, 2))
    WB = np.zeros((80, K_TOP, 256), np.float32)
    WB[0:65] = Wc0r[:, 128:193, :].transpose(1, 0, 2)
    sc_full = np.asarray(inp["gamma"], np.float32) * np.float32(
        1.0 / np.sqrt(1.0 + BN_EPS))
    be_full = (np.asarray(inp["beta"], np.float32)
               + np.asarray(inp["bc0"], np.float32) * sc_full)
    sc = np.ascontiguousarray(sc_full.reshape(2, 128).T)
    be = np.ascontiguousarray(be_full.reshape(2, 128).T)
    Wc1 = np.asarray(inp["Wc1"], np.float32)
    Wc1s = np.ascontiguousarray(Wc1.reshape(2, 128, 128).transpose(1, 0, 2))
    gbase = np.array([[_slot_base(g)] for g in range(13)], np.float32)
    return {
        "WA": WA, "WB": WB, "sc": sc, "be": be, "Wc1s": Wc1s,
        "bc1": np.asarray(inp["bc1"], np.float32).reshape(128, 1),
        "Wc2": np.asarray(inp["Wc2"], np.float32),
        "bc2": np.asarray(inp["bc2"], np.float32).reshape(64, 1),
        "Wc3": np.asarray(inp["Wc3"], np.float32),
        "bc3": np.asarray(inp["bc3"], np.float32).reshape(1, 1),
        "gbase": gbase,
    }


def _build(meta):
    import concourse.bass as bass
    import concourse.bacc as bacc
    import concourse.mybir as mybir
    from concourse import tile
    from concourse.masks import make_identity

    f32 = mybir.dt.float32
    AF = mybir.ActivationFunctionType
    Dlo, Dhi = meta["Dlo"], meta["Dhi"]
    chunks_lo, chunks_hi = meta["chunks_lo"], meta["chunks_hi"]
    cmax = meta["cmax"]
    ncl = sum(c for _, c in chunks_lo)
    nch = sum(c for _, c in chunks_hi)

    nc = bacc.Bacc("TRN2", target_bir_lowering=False, debug=False,
                   num_devices=NCORES, num_swdge_queues=4)

    u_nm0_d = nc.dram_tensor("u_nm0", [128, NROUND, F], f32, kind="ExternalInput")
    ilo_d = nc.dram_tensor("idx_lo", [128, ncl * 8], mybir.dt.int16, kind="ExternalInput")
    ihi_d = nc.dram_tensor("idx_hi", [128, nch * 8], mybir.dt.int16, kind="ExternalInput")
    dinv_d = nc.dram_tensor("dinv_nm", [128, NROUND], f32, kind="ExternalInput")
    u0tab_d = nc.dram_tensor("u0_tab", [NTOT, F], f32, kind="ExternalInput")
    W_d = [nc.dram_tensor(f"W{i}", [F, F if i < 3 else 1], f32, kind="ExternalInput") for i in range(4)]
    b_d = [nc.dram_tensor(f"b{i}", [F if i < 3 else 1, 1], f32, kind="ExternalInput") for i in range(4)]
    WA_d = nc.dram_tensor("WA", [128, K_TOP, 256], f32, kind="ExternalInput")
    WB_d = nc.dram_tensor("WB", [80, K_TOP, 256], f32, kind="ExternalInput")
    sc_d = nc.dram_tensor("sc", [128, 2], f32, kind="ExternalInput")
    be_d = nc.dram_tensor("be", [128, 2], f32, kind="ExternalInput")
    Wc1_d = nc.dram_tensor("Wc1s", [128, 2, 128], f32, kind="ExternalInput")
    bc1_d = nc.dram_tensor("bc1", [128, 1], f32, kind="ExternalInput")
    Wc2_d = nc.dram_tensor("Wc2", [128, F], f32, kind="ExternalInput")
    bc2_d = nc.dram_tensor("bc2", [F, 1], f32, kind="ExternalInput")
    Wc3_d = nc.dram_tensor("Wc3", [F, 1], f32, kind="ExternalInput")
    bc3_d = nc.dram_tensor("bc3", [1, 1], f32, kind="ExternalInput")
    gb_d = nc.dram_tensor("gbase", [13, 1], f32, kind="ExternalInput")
    out_d = nc.dram_tensor("out", [1, 13], f32, kind="ExternalOutput")
    h3dbg_d = nc.dram_tensor("h3dbg", [1, SHARD], f32, kind="ExternalOutput")

    bounceA = nc.dram_tensor("bounceA", [A_SLOTS, F], f32, kind="Internal")
    bounceB = nc.dram_tensor("bounceB", [B_SLOTS, F], f32, kind="Internal")
    u_all = nc.dram_tensor("u_all", [NTOT, F], f32, kind="Internal",
                           addr_space="Shared")
    d6656 = nc.dram_tensor("d6656", [1, SHARD], f32, kind="ExternalOutput")
    d208 = nc.dram_tensor("d208", [1, 208], mybir.dt.int16, kind="Internal")

    with tile.TileContext(nc) as tc:
        with (
            tc.tile_pool(name="persist", bufs=1) as pp,
            tc.tile_pool(name="glo", bufs=3) as gp_lo,
            tc.tile_pool(name="ghi", bufs=3) as gp_hi,
            tc.tile_pool(name="sfm", bufs=2) as sp,
            tc.tile_pool(name="psum_t", bufs=3, space="PSUM") as ps_t,
            tc.tile_pool(name="psum_w", bufs=2, space="PSUM") as ps_w,
        ):
            ident = pp.tile([128, 128], f32)
            make_identity(nc, ident[:])
            ilo = pp.tile([128, ncl * 8], mybir.dt.int16)
            nc.sync.dma_start(ilo[:], ilo_d[:])
            ihi = pp.tile([128, nch * 8], mybir.dt.int16)
            nc.sync.dma_start(ihi[:], ihi_d[:])
            dinv = pp.tile([128, NROUND], f32)
            nc.sync.dma_start(dinv[:], dinv_d[:])
            Ws, bs = [], []
            for i in range(4):
                w = pp.tile([F, F if i < 3 else 1], f32, name=f"W{i}s")
                nc.sync.dma_start(w[:], W_d[i][:])
                Ws.append(w)
                bb = pp.tile([F if i < 3 else 1, 1], f32, name=f"b{i}s")
                nc.sync.dma_start(bb[:], b_d[i][:])
                bs.append(bb)
            hcatA = pp.tile([128, SHARD], f32)
            hcatB = pp.tile([80, SHARD], f32)
            u_nm = pp.tile([128, NROUND, F], f32, name="u_nm")
            nc.sync.dma_start(u_nm[:], u_nm0_d[:])
            fold_lo = pp.tile([128, NROUND, F], f32, name="fold_lo")
            fold_hi = pp.tile([128, NROUND, F], f32, name="fold_hi")
            s_nm = fold_hi

            for layer in range(4):
                tab = u0tab_d if layer == 0 else u_all

                # ---- batched gathers + per-slab strided reduce ----
                for (chunks, D, idxt, gp, fold, lo) in (
                    (chunks_lo, Dlo, ilo, gp_lo, fold_lo, True),
                    (chunks_hi, Dhi, ihi, gp_hi, fold_hi, False),
                ):
                    view = tab[0:WIN, :] if lo else tab[HI_BASE:NTOT, :]
                    colbase = 0
                    qn = 0
                    for ci, (slabs, cols) in enumerate(chunks):
                        if cols == 0:
                            continue
                        buf = gp.tile([128, cmax, F], f32,
                                      tag="glo" if lo else "ghi",
                                      name=f"g{layer}_{int(lo)}_{ci}")
                        nidx = 128 * cols
                        nc.gpsimd.dma_gather(
                            buf[:, 0:cols, :], view,
                            idxt[:, colbase * 8:(colbase + cols) * 8],
                            nidx, nidx, F, single_packet=False,
                            queue_num=(ci * 2 + (0 if lo else 1)) % 4)
                        o = 0
                        for k in slabs:
                            d = int(D[k])
                            if d == 0:
                                nc.vector.memset(fold[:, k, :], 0.0)
                                continue
                            nc.vector.tensor_reduce(
                                out=fold[:, k, :],
                                in_=buf[:, o:o + d, :].rearrange("p d f -> p f d"),
                                axis=mybir.AxisListType.X,
                                op=mybir.AluOpType.add)
                            o += d
                        colbase += cols

                # ---- s = (fold_lo + fold_hi + u) * dinv ----
                nc.vector.tensor_tensor(
                    out=fold_hi[:], in0=fold_lo[:], in1=fold_hi[:],
                    op=mybir.AluOpType.add)
                nc.vector.tensor_tensor(
                    out=fold_lo[:], in0=fold_hi[:], in1=u_nm[:],
                    op=mybir.AluOpType.add)
                for k in range(NROUND):
                    nc.vector.tensor_scalar(
                        out=s_nm[:, k, :], in0=fold_lo[:, k, :],
                        scalar1=dinv[:, k:k + 1], scalar2=None,
                        op0=mybir.AluOpType.mult)

                # ---- transpose chunks + feature transform (+ staging) ----
                fo = F if layer < 3 else 1
                hout = (hcatA[0:64, :] if layer == 0 else
                        hcatA[64:128, :] if layer == 1 else
                        hcatB[0:64, :] if layer == 2 else
                        hcatB[64:65, :])
                hprev_next = (hcatA[0:64, :] if layer == 0 else
                              hcatA[64:128, :] if layer == 1 else
                              hcatB[0:64, :])
                ident64 = (ident[64:128, 64:128] if layer == 1
                           else ident[0:64, 0:64])
                for t in range(13):
                    sfm = sp.tile([F, 512], f32, tag="sfm", name=f"sf{layer}_{t}")
                    for kk in range(4):
                        k = t * 4 + kk
                        ptile = ps_t.tile([128, 128], f32, tag="tr",
                                          name=f"pt{layer}_{k}")
                        nc.tensor.matmul(ptile[0:F, :], s_nm[:, k, :], ident[:],
                                         is_transpose=True)
                        nc.scalar.activation(
                            sfm[:, kk * 128:(kk + 1) * 128], ptile[0:F, :],
                            AF.Copy)
                    pw = ps_w.tile([fo, 512], f32, tag="pw", name=f"pw{layer}_{t}")
                    nc.tensor.matmul(pw[:], Ws[layer][:], sfm[:],
                                     start=True, stop=True)
                    nc.scalar.activation(
                        hout[:, t * 512:(t + 1) * 512], pw[:], AF.Tanh,
                        bias=bs[layer][:])
                    if layer < 3:
                        # stage u for next layer from this chunk
                        for kk in range(4):
                            k = t * 4 + kk
                            pt2 = ps_t.tile([128, 128], f32, tag="tr",
                                            name=f"ps{layer}_{k}")
                            nc.tensor.matmul(
                                pt2[:, 0:F],
                                hout[:, k * 128:(k + 1) * 128],
                                ident64, is_transpose=True)
                            nc.scalar.activation(
                                u_nm[:, k, :], pt2[:, 0:F], AF.Copy,
                                scale=dinv[:, k:k + 1])

                if layer < 3:
                    nc.sync.dma_start(
                        bounceA.ap().rearrange("(t p) f -> p t f", p=128),
                        u_nm[:, 0:A_SLOTS // 128, :])
                    nc.sync.dma_start(
                        bounceB.ap().rearrange("(t p) f -> p t f", p=128),
                        u_nm[:, A_SLOTS // 128:NROUND, :])
                    nc.gpsimd.collective_compute(
                        "AllGather", mybir.AluOpType.bypass,
                        replica_groups=[list(range(NCORES))],
                        ins=[bounceA.ap()],
                        outs=[u_all.ap()[0:NCORES * A_SLOTS, :]])
                    nc.gpsimd.collective_compute(
                        "AllGather", mybir.AluOpType.bypass,
                        replica_groups=[list(range(NCORES))],
                        ins=[bounceB.ap()],
                        outs=[u_all.ap()[NCORES * A_SLOTS:NTOT, :]])

            # ================= sort-pool + classifier =================
            with tc.tile_pool(name="poolp", bufs=1) as qp:
                h3 = hcatB[64:65, :]
                nc.sync.dma_start(d6656.ap(), h3)
                h3g = qp.tile([13, NPG], f32)
                nc.sync.dma_start(
                    h3g[0:8, :],
                    d6656.ap()[:, 0:8 * NPG].rearrange(
                        "one (g i) -> (one g) i", g=8))
                nc.sync.dma_start(
                    h3g[8:13, :],
                    d6656.ap()[:, A_SLOTS:A_SLOTS + 5 * NPG].rearrange(
                        "one (g i) -> (one g) i", g=5))

                m8a = qp.tile([13, 8], f32)
                i8a = qp.tile([13, 8], mybir.dt.uint32)
                nc.vector.max(m8a[:], h3g[:])
                nc.vector.max_index(i8a[:], m8a[:], h3g[:])
                h3m = qp.tile([13, NPG], f32)
                nc.vector.match_replace(h3m[:], m8a[:], h3g[:], -2.0)
                m8b = qp.tile([13, 8], f32)
                i8b = qp.tile([13, 8], mybir.dt.uint32)
                nc.vector.max(m8b[:], h3m[:])
                nc.vector.max_index(i8b[:], m8b[:], h3m[:])

                idx2d = qp.tile([13, 16], f32)
                nc.vector.memset(idx2d[:], 0.0)
                nc.vector.tensor_copy(idx2d[:, 0:8], i8a[:])
                nc.vector.tensor_copy(idx2d[:, 8:15], i8b[:, 0:7])
                gbase = qp.tile([13, 1], f32)
                nc.sync.dma_start(gbase[:], gb_d[:])
                nc.vector.tensor_scalar(
                    out=idx2d[:], in0=idx2d[:], scalar1=gbase[:, 0:1],
                    scalar2=None, op0=mybir.AluOpType.add)
                idx16 = qp.tile([13, 16], mybir.dt.int16)
                nc.vector.tensor_copy(idx16[:], idx2d[:])
                nc.sync.dma_start(
                    d208.ap().rearrange("one (g r) -> (one g) r", g=13),
                    idx16[:])
                idx128 = qp.tile([128, 13], mybir.dt.int16)
                for kk in range(8):
                    nc.sync.dma_start(
                        idx128[kk * 16:(kk + 1) * 16, :],
                        d208.ap().rearrange("one (s p) -> (one p) s", p=16))

                poolA = qp.tile([128, 208], f32)
                nc.gpsimd.ap_gather(poolA[:], hcatA[:], idx128[:],
                                    channels=128, num_elems=SHARD, d=1,
                                    num_idxs=208)
                poolB = qp.tile([80, 208], f32)
                nc.gpsimd.ap_gather(poolB[:], hcatB[:], idx128[0:80, :],
                                    channels=80, num_elems=SHARD, d=1,
                                    num_idxs=208)

                WA = qp.tile([128, K_TOP, 256], f32)
                nc.sync.dma_start(WA[:], WA_d[:])
                WB = qp.tile([80, K_TOP, 256], f32)
                nc.sync.dma_start(WB[:], WB_d[:])
                sc = qp.tile([128, 2], f32)
                nc.sync.dma_start(sc[:], sc_d[:])
                be = qp.tile([128, 2], f32)
                nc.sync.dma_start(be[:], be_d[:])
                Wc1s = qp.tile([128, 2, 128], f32)
                nc.sync.dma_start(Wc1s[:], Wc1_d[:])
                bc1 = qp.tile([128, 1], f32)
                nc.sync.dma_start(bc1[:], bc1_d[:])
                Wc2 = qp.tile([128, F], f32)
                nc.sync.dma_start(Wc2[:], Wc2_d[:])
                bc2 = qp.tile([F, 1], f32)
                nc.sync.dma_start(bc2[:], bc2_d[:])
                Wc3 = qp.tile([F, 1], f32)
                nc.sync.dma_start(Wc3[:], Wc3_d[:])
                bc3 = qp.tile([1, 1], f32)
                nc.sync.dma_start(bc3[:], bc3_d[:])

                z1 = []
                for mh in range(2):
                    pz = ps_w.tile([128, 13], f32, tag="pw", name=f"pz{mh}")
                    first = True
                    for r in range(K_TOP):
                        nc.tensor.matmul(
                            pz[:], WA[:, r, mh * 128:(mh + 1) * 128],
                            poolA[:, r:r + 16 * 12 + 1:16],
                            start=first, stop=False)
                        first = False
                        nc.tensor.matmul(
                            pz[:], WB[0:65, r, mh * 128:(mh + 1) * 128],
                            poolB[0:65, r:r + 16 * 12 + 1:16],
                            start=False, stop=(r == K_TOP - 1))
                    zz = qp.tile([128, 13], f32, tag=f"z1_{mh}", name=f"z1_{mh}")
                    nc.scalar.activation(zz[:], pz[:], AF.Relu,
                                         bias=be[:, mh:mh + 1],
                                         scale=sc[:, mh:mh + 1])
                    z1.append(zz)
                pz2 = ps_w.tile([128, 13], f32, tag="pw", name="pz2")
                nc.tensor.matmul(pz2[:], Wc1s[:, 0, :], z1[0][:],
                                 start=True, stop=False)
                nc.tensor.matmul(pz2[:], Wc1s[:, 1, :], z1[1][:],
                                 start=False, stop=True)
                z2 = qp.tile([128, 13], f32)
                nc.scalar.activation(z2[:], pz2[:], AF.Relu, bias=bc1[:])
                pz3 = ps_w.tile([F, 13], f32, tag="pw", name="pz3")
                nc.tensor.matmul(pz3[:], Wc2[:], z2[:], start=True, stop=True)
                z3 = qp.tile([F, 13], f32)
                nc.scalar.activation(z3[:], pz3[:], AF.Relu, bias=bc2[:])
                pz4 = ps_w.tile([1, 13], f32, tag="pw", name="pz4")
                nc.tensor.matmul(pz4[:], Wc3[:], z3[:], start=True, stop=True)
                zf = qp.tile([1, 13], f32)
                nc.vector.tensor_scalar(out=zf[:], in0=pz4[:],
                                        scalar1=bc3[0:1, 0:1], scalar2=None,
                                        op0=mybir.AluOpType.add)
                nc.sync.dma_start(out_d[:], zf[:])
                nc.sync.dma_start(h3dbg_d[:], h3)

    nc.compile()
    return nc


def kernel(**inputs):
    from concourse import bass_utils

    x = np.asarray(inputs["x"], np.float32)
    edge_index = np.asarray(inputs["edge_index"])

    meta, idx_lo, idx_hi, u0_tab, u_nm0, dinv_nms = _prep(x, edge_index)
    key = ("prog", tuple(meta["Dlo"]), tuple(meta["Dhi"]))
    if key in _CACHE:
        nc = _CACHE[key]
    else:
        nc = _build(meta)
        _CACHE[key] = nc

    cw = _pack_classifier(inputs)
    in_maps = []
    for c in range(NCORES):
        m = {
            "u_nm0": u_nm0[c],
            "idx_lo": idx_lo[c],
            "idx_hi": idx_hi[c],
            "dinv_nm": dinv_nms[c],
            "u0_tab": u0_tab,
            "WA": cw["WA"], "WB": cw["WB"], "sc": cw["sc"], "be": cw["be"],
            "Wc1s": cw["Wc1s"], "bc1": cw["bc1"], "Wc2": cw["Wc2"],
            "bc2": cw["bc2"], "Wc3": cw["Wc3"], "bc3": cw["bc3"],
            "gbase": cw["gbase"],
        }
        for i in range(4):
            m[f"W{i}"] = np.asarray(inputs[f"W{i}"], np.float32).reshape(
                F, F if i < 3 else 1)
            m[f"b{i}"] = np.asarray(inputs[f"b{i}"], np.float32).reshape(
                F if i < 3 else 1, 1)
        in_maps.append(m)

    trace = os.environ.get("KERNEL_TRACE", "0") == "1"
    kwargs = {}
    if trace:
        import sys, types
        if "antenv.axon_hooks" not in sys.modules:
            sys.path.insert(0, "/root/.axon_site")
            from trn_agent_boot.trn_boot import _ntff_profile_via_ctypes
            mm = types.ModuleType("antenv.axon_hooks")
            mm.get_axon_ntff_profile_hook = (
                lambda: _ntff_profile_via_ctypes("/opt/axon/libaxon_pjrt.so"))
            sys.modules["antenv.axon_hooks"] = mm
        import tempfile
        kwargs = dict(trace=True, tmpdir=tempfile.mkdtemp())

    res = bass_utils.run_bass_kernel_spmd(
        nc, in_maps, core_ids=list(range(NCORES)), **kwargs)

    global LAST_EXEC_NS, LAST_H3
    LAST_EXEC_NS = res.exec_time_ns
    LAST_H3 = [res.results[c]["h3dbg"] for c in range(NCORES)]

    out = np.zeros((G, 1), np.float32)
    for c in range(NCORES):
        ngr = GRAPHS_PER_CORE[c]
        out[GSTART[c]:GSTART[c] + ngr, 0] = res.results[c]["out"][0, :ngr]
    return out


LAST_EXEC_NS = None
LAST_H3 = None
